# revision 19
# baseline (speedup 1.0000x reference)
"""Causal multi-head attention (B=2, S=2048, D=1024, 16 heads of 64) on 8 TRN2
NeuronCores.

Sharding: core c -> batch b = c//4, head-group g = c%4 (4 heads = 256 model
dims per core).  Wq/Wk/Wv column-parallel, Wo row-parallel; the 4 partial
outputs per batch are summed on the host (no collectives).

Per-core data flow (matmul compute in bf16, fp32 PSUM accumulation):
  QT = (Wq_g/8) @ x^T      [256, 2048]   (1/sqrt(hd) folded into Wq,bq)
  KT = Wk_g @ x^T          [256, 2048]
  V  = x @ Wv_g^T + bv     [2048, 256]   natural layout, ones-augmented
  attention per head pair (64-row PE tiling throughout):
    ST[sk,sq] = K_h @ Q_h^T          two heads at row groups 0/64
    P = exp(ST + causal mask)        ACT, one [128,<=512] exp per sk tile
    AV: two K=64 halves accumulate in separate PSUM banks (row-tile bank
        rule); lhsT is ones-augmented V so row 64 = l[sq]
    preoutT = (poA+poB)[0:64] * 1/(lA+lB)
  out_partial = preoutT.T @ Wo_g^T   [2048, 1024] fp32

The emission is a hand-interleaved static schedule: score units (which feed
the scalar engine's exp stream, the latency-critical resource) alternate
with single-PSUM-bank projection/output/AV "filler" units so no engine
starves and PSUM slots rotate freely.  AV of block N is emitted inside
block N+1's score stream.
Host: out[b] = sum of the 4 head-group partials + bo.
"""

import numpy as np
import ml_dtypes

B, S, D = 2, 2048, 1024
HD = 64
NH = D // HD
N_CORES = 8
GROUPS = 4          # head-groups (tensor-parallel)
JG = D // GROUPS    # local dims per core = 256
NHL = JG // HD      # local heads = 4
KCH = D // 128      # contraction chunks for projections = 8
NKT = S // 128      # sk tiles = 16
NJB = S // 512      # query blocks of 512 = 4
MASK_VAL = -1e9

BF16 = ml_dtypes.bfloat16

_cached = {}


def _build():
    import concourse.bacc as bacc
    import concourse.tile as tile
    import concourse.mybir as mybir

    f32 = mybir.dt.float32
    bf16 = mybir.dt.bfloat16
    Exp = mybir.ActivationFunctionType.Exp

    nc = bacc.Bacc("TRN2", target_bir_lowering=False, debug=False,
                   num_devices=N_CORES)

    xT = nc.dram_tensor("xT", [128, KCH, S], bf16, kind="ExternalInput").ap()
    wqT = nc.dram_tensor("wqT", [128, KCH, JG], bf16, kind="ExternalInput").ap()
    wkT = nc.dram_tensor("wkT", [128, KCH, JG], bf16, kind="ExternalInput").ap()
    wvT = nc.dram_tensor("wvT", [128, KCH, JG], bf16, kind="ExternalInput").ap()
    woT = nc.dram_tensor("woT", [128, 2, D], bf16, kind="ExternalInput").ap()
    bqc = nc.dram_tensor("bqc", [JG, 1], f32, kind="ExternalInput").ap()
    bkc = nc.dram_tensor("bkc", [JG, 1], f32, kind="ExternalInput").ap()
    bvb = nc.dram_tensor("bvb", [128, JG], f32, kind="ExternalInput").ap()
    maskT = nc.dram_tensor("maskT", [128, 128], bf16, kind="ExternalInput").ap()
    out = nc.dram_tensor("out", [S, D], f32, kind="ExternalOutput").ap()

    with tile.TileContext(nc) as tc:
        with (
            tc.tile_pool(name="const", bufs=1) as cpool,
            tc.tile_pool(name="pbig", bufs=4) as p_pool,
            tc.tile_pool(name="small", bufs=4) as small_pool,
            tc.tile_pool(name="outp", bufs=3) as out_pool,
            tc.tile_pool(name="mm_ps", bufs=4, space="PSUM") as mm_ps,
            tc.tile_pool(name="po_ps", bufs=4, space="PSUM") as po_ps,
        ):
            # ---- input DMAs (contiguous, host pre-tiled) ----
            wq_sb = cpool.tile([128, KCH, JG], bf16)
            nc.sync.dma_start(wq_sb[:], wqT[:])
            bq_sb = cpool.tile([128, 2], f32)
            nc.sync.dma_start(bq_sb[:], bqc.rearrange("(t p) o -> p (t o)", p=128))
            xt_k = []
            for k in range(KCH):
                t_ = cpool.tile([128, S], bf16, name=f"xt{k}")
                xt_k.append(t_)
            nc.sync.dma_start(xt_k[0][:], xT[:, 0, :])
            nc.sync.dma_start(xt_k[1][:], xT[:, 1, :])
            wk_sb = cpool.tile([128, KCH, JG], bf16)
            nc.sync.dma_start(wk_sb[:], wkT[:])
            bk_sb = cpool.tile([128, 2], f32)
            nc.sync.dma_start(bk_sb[:], bkc.rearrange("(t p) o -> p (t o)", p=128))
            mask_sb = cpool.tile([128, 128], bf16)
            nc.sync.dma_start(mask_sb[:], maskT[:])
            for k in range(2, KCH):
                nc.sync.dma_start(xt_k[k][:], xT[:, k, :])
            wv_sb = cpool.tile([128, KCH, JG], bf16)
            nc.sync.dma_start(wv_sb[:], wvT[:])
            bvb_sb = cpool.tile([128, JG], f32)
            nc.sync.dma_start(bvb_sb[:], bvb[:])
            wo_sb = cpool.tile([128, 2, D], bf16)
            nc.sync.dma_start(wo_sb[:], woT[:])

            qt = [cpool.tile([128, S], bf16, name=f"qt{t}") for t in range(2)]
            kt = [cpool.tile([128, S], bf16, name=f"kt{t}") for t in range(2)]
            v_all = cpool.tile([128, NKT, NHL * 65], bf16)
            nc.vector.memset(
                v_all.rearrange("p k (h c) -> p k h c", c=65)[:, :, :, 64:65], 1.0)
            po = [cpool.tile([128, S], bf16, name=f"po{t}") for t in range(2)]

            # warm the ACT exp table while DMAs run
            warm = small_pool.tile([1, 4], f32, tag="lrow")
            nc.vector.memset(warm[:], 0.0)
            nc.scalar.activation(warm[:], warm[:], Exp)

            # ---- single-PSUM-bank work units ----
            def u_proj_qk(w_sb, b_sb, dst, t, n):
                ps = mm_ps.tile([128, 512], f32, tag="mm",
                                name=f"pp{id(w_sb) % 97}_{t}_{n}")
                for k in range(KCH):
                    nc.tensor.matmul(
                        ps[:], lhsT=w_sb[:, k, 128 * t:128 * t + 128],
                        rhs=xt_k[k][:, 512 * n:512 * n + 512],
                        start=(k == 0), stop=(k == KCH - 1))
                nc.vector.tensor_scalar_add(
                    dst[t][:, 512 * n:512 * n + 512], ps[:], b_sb[:, t:t + 1])

            def u_proj_v(si):
                ps = mm_ps.tile([128, 512], f32, tag="mm", name=f"pv{si}")
                for k in range(KCH):
                    nc.tensor.matmul(
                        ps[:, 0:256],
                        lhsT=xt_k[k][:, 128 * si:128 * si + 128],
                        rhs=wv_sb[:, k, :],
                        start=(k == 0), stop=(k == KCH - 1))
                nc.vector.tensor_add(
                    v_all[:, si, :].rearrange(
                        "p (h c) -> p h c", c=65)[:, :, 0:64],
                    ps[:, 0:256].rearrange("p (h c) -> p h c", c=64),
                    bvb_sb.rearrange("p (h c) -> p h c", c=64))

            def u_wo(m, n):
                ps = mm_ps.tile([128, 512], f32, tag="mm", name=f"pw{m}_{n}")
                for t in range(2):
                    nc.tensor.matmul(
                        ps[:], lhsT=po[t][:, 128 * m:128 * m + 128],
                        rhs=wo_sb[:, t, 512 * n:512 * n + 512],
                        start=(t == 0), stop=(t == 1))
                ob = out_pool.tile([128, 512], f32, tag="ob")
                nc.vector.tensor_copy(ob[:], ps[:])
                nc.sync.dma_start(
                    out[128 * m:128 * m + 128, 512 * n:512 * n + 512], ob[:])

            def u_scores(pair, j, ki, pt):
                d = max(0, 128 * ki - 512 * j)
                sts = [mm_ps.tile([128, 512], f32, tag="mm",
                                  name=f"st{pair}_{j}_{ki}_{hh}")
                       for hh in range(2)]
                for hh in range(2):
                    base = 64 * hh
                    nc.tensor.matmul(
                        sts[hh][:, d:512],
                        lhsT=kt[pair][base:base + 64, 128 * ki:128 * ki + 128],
                        rhs=qt[pair][base:base + 64,
                                     512 * j + d:512 * j + 512],
                        start=True, stop=True)
                for hh in range(2):
                    nc.scalar.activation(
                        pt[hh][:, ki, d:512], sts[hh][:, d:512], Exp)
                    if ki >= 4 * j:
                        nc.gpsimd.tensor_mul(
                            pt[hh][:, ki, d:d + 128],
                            pt[hh][:, ki, d:d + 128], mask_sb[:])

            def u_av(pair, j, ki, pt, pos, nk):
                d = max(0, 128 * ki - 512 * j)
                for hh in range(2):
                    h = 2 * pair + hh
                    for half in range(2):  # row groups alternate 0/64
                        pb = 64 * half
                        nc.tensor.matmul(
                            pos[2 * hh + half][0:65, d:512],
                            lhsT=v_all[pb:pb + 64, ki, 65 * h:65 * h + 65],
                            rhs=pt[hh][pb:pb + 64, ki, d:512],
                            start=(ki == 0), stop=(ki == nk - 1))

            def u_evac(pair, j, hh, pos):
                poA, poB = pos[2 * hh], pos[2 * hh + 1]
                bsb = small_pool.tile([65, 512], f32, tag="bsb")
                nc.vector.tensor_copy(bsb[:], poB[:])
                ssum = small_pool.tile([65, 512], f32, tag="ssum")
                nc.vector.tensor_add(ssum[:], poA[:], bsb[:])
                lrow = small_pool.tile([1, 512], f32, tag="lrow")
                nc.vector.tensor_copy(lrow[:], ssum[64:65, :])
                rb = small_pool.tile([64, 512], f32, tag="rb")
                nc.gpsimd.partition_broadcast(rb[:], lrow[:])
                rbr = small_pool.tile([64, 512], f32, tag="rbr")
                nc.vector.reciprocal_approx_fast(rbr[:], rb[:])
                nc.vector.tensor_mul(
                    po[pair][64 * hh:64 * hh + 64, 512 * j:512 * j + 512],
                    ssum[0:64, :], rbr[:])

            # ---- static interleaved schedule ----
            # ends with the small j=0 blocks so the non-overlappable AV+evac
            # tail is as short as possible
            block_order = [(0, 1), (1, 1), (0, 2), (1, 2),
                           (0, 3), (1, 3), (0, 0), (1, 0)]
            # filler units with their emission prerequisites handled by list
            # order; consumed between score units to keep the PE busy while
            # the exp stream paces the scalar engine.
            filler = [
                lambda: u_proj_qk(wq_sb, bq_sb, qt, 1, 1),
                lambda: u_proj_qk(wk_sb, bk_sb, kt, 1, 0),
                lambda: u_proj_qk(wk_sb, bk_sb, kt, 1, 1),
                lambda: u_proj_v(0), lambda: u_proj_v(1),
                lambda: u_proj_v(2), lambda: u_proj_v(3),
                lambda: u_proj_v(4), lambda: u_proj_v(5),
                lambda: u_proj_v(6), lambda: u_proj_v(7),
                lambda: u_proj_qk(wq_sb, bq_sb, qt, 0, 2),
                lambda: u_proj_qk(wk_sb, bk_sb, kt, 0, 2),
                lambda: u_proj_qk(wq_sb, bq_sb, qt, 1, 2),
                lambda: u_proj_qk(wk_sb, bk_sb, kt, 1, 2),
                lambda: u_proj_v(8), lambda: u_proj_v(9),
                lambda: u_proj_v(10), lambda: u_proj_v(11),
                lambda: u_proj_qk(wq_sb, bq_sb, qt, 0, 3),
                lambda: u_proj_qk(wk_sb, bk_sb, kt, 0, 3),
                lambda: u_proj_qk(wq_sb, bq_sb, qt, 1, 3),
                lambda: u_proj_qk(wk_sb, bk_sb, kt, 1, 3),
                lambda: u_proj_v(12), lambda: u_proj_v(13),
                lambda: u_proj_v(14), lambda: u_proj_v(15),
                lambda: u_proj_qk(wq_sb, bq_sb, qt, 0, 0),
                lambda: u_proj_qk(wq_sb, bq_sb, qt, 1, 0),
            ]
            # minimum emission prefix for each block's scores: (needed filler
            # count that must be emitted before that block's first score unit)
            # block (pair,j) scores need qt[pair] n=j and kt[pair] n<=j;
            # AV(pair,j) (emitted a block later) needs v tiles si < 4(j+1).
            prefix_needed = {
                (0, 1): 0,   # prelude covers q0n1/k0n0/k0n1
                (1, 1): 11,  # q1n1,k1n0,k1n1 + v0-7 (AV of (0,1) runs here)
                (0, 2): 13,
                (1, 2): 19,  # q1n2,k1n2 + v8-11 (AV of (0,2) runs here)
                (0, 3): 21,
                (1, 3): 27,  # q1n3,k1n3 + v12-15 (AV of (0,3) runs here)
                (0, 0): 28,
                (1, 0): 29,
            }
            wo_ready = []

            # prelude: just enough to start block (0,1) scores
            u_proj_qk(wq_sb, bq_sb, qt, 0, 1)
            u_proj_qk(wk_sb, bk_sb, kt, 0, 0)
            u_proj_qk(wk_sb, bk_sb, kt, 0, 1)

            fill_i = 0
            fill_carry = [0.0]
            total_scores = sum(4 * (j + 1) for _, j in block_order)
            scores_done = 0
            prev = None  # (pair, j, pt, pos, nk, next_av_ki)

            def pop_filler(n=1):
                nonlocal fill_i
                for _ in range(n):
                    if wo_ready:
                        wo_ready.pop(0)()
                    elif fill_i < len(filler):
                        filler[fill_i]()
                        fill_i += 1

            for bi, (pair, j) in enumerate(block_order):
                nk = 4 * (j + 1)
                # make sure this block's q/k (and prev block's v) are emitted
                while fill_i < prefix_needed[(pair, j)]:
                    filler[fill_i]()
                    fill_i += 1
                pt = [p_pool.tile([128, NKT, 512], bf16, tag="p",
                                  name=f"pt{pair}_{j}_{hh}")
                      for hh in range(2)]
                prev_av = []
                if prev is not None:
                    ppair, pj, ppt, ppos, pnk = prev
                    prev_av = [(ppair, pj, k2, ppt, ppos, pnk)
                               for k2 in range(pnk)]
                for ki in range(nk):
                    u_scores(pair, j, ki, pt)
                    scores_done += 1
                    # drain a slice of the previous block's AV
                    n_av = (len(prev_av) + nk - ki - 1) // (nk - ki)
                    for _ in range(min(n_av, len(prev_av))):
                        u_av(*prev_av.pop(0))
                    # rate-matched filler consumption (even spread)
                    rem_scores = max(1, total_scores - scores_done)
                    rem_fill = (len(filler) - fill_i) + len(wo_ready)
                    fill_carry[0] += rem_fill / rem_scores
                    while fill_carry[0] >= 1.0:
                        fill_carry[0] -= 1.0
                        pop_filler(1)
                for u in prev_av:
                    u_av(*u)
                if prev is not None:
                    ppair, pj, ppt, ppos, pnk = prev
                    u_evac(ppair, pj, 0, ppos)
                    u_evac(ppair, pj, 1, ppos)
                    if ppair == 1:
                        wo_ready.extend(
                            [lambda m=m, n=n: u_wo(m, n)
                             for m in range(4 * pj, 4 * pj + 4)
                             for n in range(2)])
                pos = [po_ps.tile([65, 512], f32, tag="po",
                                  name=f"pos{pair}_{j}_{hh}_{half}")
                       for hh in range(2) for half in range(2)]
                prev = (pair, j, pt, pos, nk)

            # tail: AV + evac of the final block, then remaining output proj
            ppair, pj, ppt, ppos, pnk = prev
            for k2 in range(pnk):
                u_av(ppair, pj, k2, ppt, ppos, pnk)
                pop_filler(1)
            u_evac(ppair, pj, 0, ppos)
            u_evac(ppair, pj, 1, ppos)
            wo_ready.extend([lambda m=m, n=n: u_wo(m, n)
                             for m in range(4 * pj, 4 * pj + 4)
                             for n in range(2)])
            while fill_i < len(filler) or wo_ready:
                pop_filler(1)

    nc.compile()
    return nc


def _get_nc():
    if "nc" not in _cached:
        _cached["nc"] = _build()
    return _cached["nc"]


def _make_in_maps(x, Wq, bq, Wk, bk, Wv, bv, Wo):
    sc = 1.0 / np.sqrt(HD)
    tri = np.arange(128)
    mask = np.where(tri[:, None] <= tri[None, :], 1.0, 0.0).astype(BF16)
    in_maps = []
    for c in range(N_CORES):
        b, g = divmod(c, GROUPS)
        sl = slice(JG * g, JG * (g + 1))

        def tile_k(a):  # [D, M] -> [128, D//128, M] contiguous
            return np.ascontiguousarray(
                a.reshape(a.shape[0] // 128, 128, a.shape[1]).transpose(1, 0, 2))

        in_maps.append({
            "xT": tile_k(x[b].T.astype(BF16)),
            "wqT": tile_k((Wq[sl] * sc).T.astype(BF16)),
            "wkT": tile_k(Wk[sl].T.astype(BF16)),
            "wvT": tile_k(Wv[sl].T.astype(BF16)),
            "woT": tile_k(Wo[:, sl].T.astype(BF16)),
            "bqc": (bq[sl] * sc).astype(np.float32).reshape(JG, 1),
            "bkc": bk[sl].astype(np.float32).reshape(JG, 1),
            "bvb": np.broadcast_to(bv[sl].astype(np.float32), (128, JG)).copy(),
            "maskT": mask,
        })
    return in_maps


def kernel(x, Wq, bq, Wk, bk, Wv, bv, Wo, bo, _return_results=False):
    from concourse.bass_utils import run_bass_kernel_spmd

    nc = _get_nc()
    in_maps = _make_in_maps(np.asarray(x, np.float32), np.asarray(Wq, np.float32),
                            np.asarray(bq, np.float32), np.asarray(Wk, np.float32),
                            np.asarray(bk, np.float32), np.asarray(Wv, np.float32),
                            np.asarray(bv, np.float32), np.asarray(Wo, np.float32))
    res = run_bass_kernel_spmd(nc, in_maps, core_ids=list(range(N_CORES)))
    full = np.empty((B, S, D), np.float32)
    for b in range(B):
        acc = res.results[4 * b]["out"].astype(np.float32).copy()
        for g in range(1, GROUPS):
            acc += res.results[4 * b + g]["out"]
        full[b] = acc + np.asarray(bo, np.float32)[None, :]
    if _return_results:
        return full, res
    return full
